# revision 16
# baseline (speedup 1.0000x reference)
"""Bass/Tile Trainium2 kernel for masked dot-product attention.

Problem: B=32 (batch*heads), S=2048, D=128, fp32.
  out = softmax(mask(Q @ K^T / sqrt(D))) @ V
  mask = key-padding (k >= valid_len[b]) OR causal (k > q).

Sharding: batch dim across 8 cores (4 batches/core), no cross-core comm.

Per-core device algorithm (per batch):
  - Q^T, K^T loaded in [D, S] layout (host pre-transposes during sharding).
  - Scores computed transposed, ST[k, q] = K @ Q^T, in 512-wide q blocks,
    k-tiles chunked 3-at-a-time into one PSUM tile [128, 1536].
  - One ACT exp instruction per chunk (scale=1/sqrt(D) fused), PSUM -> SBUF.
  - Causal mask: multiply diagonal 128x128 sub-tiles by a constant 0/1
    triangle; fully-masked sub-tiles are simply skipped in the PV matmuls.
  - Padding mask: per-partition 0/1 multiplier column (k is the partition
    dim in ST layout), one tensor_scalar mul per affected sub-tile.
  - PV: lhsT = P~ slice [k,128q], rhs = V_aug [k, 129] (V with an appended
    ones column) -> PSUM O[q, 0:128] and Z[q] at column 128 in one pass.
  - Epilogue: rz = 1/Z (DVE reciprocal), out = O * rz (tensor_scalar).
"""

import math

import numpy as np

B, S, D = 32, 2048, 128
N_CORES = 8
B_LOC = B // N_CORES  # 4 batches per core
NT = S // 128  # 16 k-tiles per batch
NJ = S // 512  # 4 q-blocks per batch
CHUNK = 3  # k-tiles per score PSUM tile ([128, 1536] = 3 banks)

_PROGRAM_CACHE = {}
_RUNNER_CACHE = {}


def _apply_tile_drain_patch():
    """walrus on this image only accepts 1 sync-wait per instruction; Tile's
    kernel-tail drain attaches every outstanding sem wait to one drain.
    Spill the excess onto dedicated single-wait NOPs (SP is FIFO, so waiting
    right after the drain and before the barrier is equivalent)."""
    import bass_rust
    import concourse.tile as tile
    from concourse.vector_clock import ScopedClock

    if getattr(tile.TileContext, "_drain_patch_applied", False):
        return

    def _patched(self, tick_clock, wait_clock):
        nc = self.nc
        drain_inst = nc.sync.drain()
        wait_clock.add_sem_waits(
            drain_inst.ins, ScopedClock({None: tick_clock.global_clock})
        )
        si = drain_inst.ins.sync_info
        if si is not None and si.on_wait and len(si.on_wait) > 1:
            waits = list(si.on_wait)
            drain_inst.ins.sync_info = bass_rust.SyncInfo(
                on_wait=[waits[0]], on_update=list(si.on_update or [])
            )
            for w in waits[1:]:
                nop = nc.sync.nop()
                nop.ins.sync_info = bass_rust.SyncInfo(on_wait=[w], on_update=[])
        nc.all_engine_barrier()
        assert self.sems is not None
        popped = nc._tile_sem_poison_stack.pop()
        assert popped is self._sem_poison
        nc.clear_and_free_semaphores(list(self.sems.allocated().values()))
        nc.all_engine_barrier()

    tile.TileContext._drain_and_barrier = _patched
    tile.TileContext._drain_patch_applied = True


def _split_multi_waits(nc):
    """walrus on this image accepts only one sync-wait command per
    instruction; Tile emits several. Move excess waits onto same-engine NOPs
    inserted immediately before the instruction (per-engine streams are
    in-order, so this is equivalent)."""
    import bass_rust
    import concourse.mybir as mybir

    for bb in nc.main_func.blocks:
        insts = bb.instructions
        out = []
        for inst in insts:
            si = inst.sync_info
            if si is not None and si.on_wait and len(si.on_wait) > 1:
                waits = list(si.on_wait)
                for w in waits[:-1]:
                    nop = mybir.InstNoOp(
                        name=f"I-{nc.next_id()}", ins=[], outs=[]
                    )
                    nop.engine = inst.engine
                    nop.sync_info = bass_rust.SyncInfo(on_wait=[w], on_update=[])
                    out.append(nop)
                inst.sync_info = bass_rust.SyncInfo(
                    on_wait=[waits[-1]], on_update=list(si.on_update or [])
                )
            out.append(inst)
        insts[:] = out


def _build_program(
    causal: bool, t_pad_start: int, reps: int = 1, fast: bool = True
):
    """Trace the per-core Bass program. t_pad_start: first k-tile index that
    may contain padding-masked keys (host-computed from min(valid_lens)).
    fast=True: QK matmuls in float32r (full-rate PE) and PV in bf16;
    fast=False: everything plain fp32 (4 cyc/row matmuls, max accuracy)."""
    import concourse.bass as bass
    import concourse.mybir as mybir
    import concourse.tile as tile
    from concourse.tile_rust import add_dep_helper

    _apply_tile_drain_patch()

    f32 = mybir.dt.float32
    f32r = mybir.dt.float32r
    QKDT = f32r if fast else f32  # Q/K dtype (f32r = full-rate PE matmul)
    PDT = mybir.dt.bfloat16 if fast else f32  # probs/V dtype
    DA = D + 1  # V augmented with a ones column

    nc = bass.Bass()
    qT = nc.dram_tensor("qT", [B_LOC, D, S], QKDT, kind="ExternalInput")
    kT = nc.dram_tensor("kT", [B_LOC, D, S], QKDT, kind="ExternalInput")
    v = nc.dram_tensor("v", [B_LOC, S, D], PDT, kind="ExternalInput")
    pad01 = nc.dram_tensor("pad01", [B_LOC, 128, NT], f32, kind="ExternalInput")
    out = nc.dram_tensor("out", [B_LOC, S, D], f32, kind="ExternalOutput")

    with tile.TileContext(nc) as tc:
        with (
            tc.tile_pool(name="const", bufs=1) as constp,
            tc.tile_pool(name="io", bufs=2) as iop,
            tc.tile_pool(name="probs", bufs=8) as probp,
            tc.tile_pool(name="outp", bufs=4) as outp,
            tc.tile_pool(name="small", bufs=4) as smallp,
            tc.tile_pool(name="spsum", bufs=2, space="PSUM") as spsum,
            tc.tile_pool(name="opsum", bufs=2, space="PSUM") as opsum,
        ):
            # 0/1 lower triangle: tri[p, c] = 1.0 iff c >= p (keep k <= q)
            tri = constp.tile([128, 128], PDT)
            nc.gpsimd.memset(tri[:], 1.0)
            nc.gpsimd.affine_select(
                out=tri[:],
                in_=tri[:],
                compare_op=mybir.AluOpType.is_ge,
                fill=0.0,
                base=0,
                pattern=[[1, 128]],
                channel_multiplier=-1,
            )

            for _rep in range(reps):
                for b in range(B_LOC):
                    qt_sb = iop.tile([128, S], QKDT, tag="qT")
                    nc.sync.dma_start(qt_sb[:], qT[b])
                    kt_sb = iop.tile([128, S], QKDT, tag="kT")
                    nc.sync.dma_start(kt_sb[:], kT[b])
                    v_sb = iop.tile([128, NT * DA], PDT, tag="v")
                    v3 = v_sb[:].rearrange("p (t d) -> p t d", d=DA)
                    nc.sync.dma_start(
                        v3[:, :, 0:D], v[b].rearrange("(t p) d -> p t d", p=128)
                    )
                    nc.vector.memset(v3[:, :, D : D + 1], 1.0)
                    pad_sb = iop.tile([128, NT], f32, tag="pad")
                    nc.sync.dma_start(pad_sb[:], pad01[b])

                    for j in range(NJ):
                        T = 4 * j + 4 if causal else NT
                        o_ps = [
                            opsum.tile([128, 2 * DA], f32, tag="o", name=f"o_ps{jj}")
                            for jj in range(2)
                        ]
                        p_tiles = {}  # chunk start -> (p_sb, chunk list)
                        for c0 in range(0, T, CHUNK):
                            ch = list(range(c0, min(c0 + CHUNK, T)))
                            L = len(ch)
                            s_ps = spsum.tile([128, L * 512], f32, tag="s")
                            for i, t in enumerate(ch):
                                nc.tensor.matmul(
                                    s_ps[:, 512 * i : 512 * (i + 1)],
                                    kt_sb[:, 128 * t : 128 * (t + 1)],
                                    qt_sb[:, 512 * j : 512 * (j + 1)],
                                    start=True,
                                    stop=True,
                                )
                            p_sb = probp.tile(
                                [128, L * 512], PDT, tag="p", name=f"p_sb{c0}"
                            )
                            nc.scalar.activation(
                                p_sb[:],
                                s_ps[:],
                                mybir.ActivationFunctionType.Exp,
                                scale=float(1.0 / math.sqrt(D)),
                            )
                            for i, t in enumerate(ch):
                                if causal and t >= 4 * j:
                                    r = t - 4 * j
                                    sl = p_sb[
                                        :, 512 * i + 128 * r : 512 * i + 128 * r + 128
                                    ]
                                    nc.vector.tensor_mul(sl, sl, tri[:])
                                if t >= t_pad_start:
                                    sl = p_sb[:, 512 * i : 512 * (i + 1)]
                                    nc.vector.tensor_scalar_mul(
                                        sl, sl, pad_sb[:, t : t + 1]
                                    )
                            p_tiles[c0] = p_sb
                        # PV: one fully-sequential accumulation group per
                        # (bank, column-half) — interleaving groups on one
                        # PSUM bank corrupts has_written state (start=True
                        # clears the whole bank).
                        prev_stop = [None, None]
                        for qt in range(4):
                            jj, m = qt // 2, qt % 2
                            kmax = (4 * j + qt) if causal else (NT - 1)
                            for t in range(kmax + 1):
                                c0 = (t // CHUNK) * CHUNK
                                i = t - c0
                                p_sb = p_tiles[c0]
                                mm = nc.tensor.matmul(
                                    o_ps[jj][:, DA * m : DA * m + DA],
                                    p_sb[
                                        :, 512 * i + 128 * qt : 512 * i + 128 * qt + 128
                                    ],
                                    v_sb[:, DA * t : DA * (t + 1)],
                                    start=(t == 0),
                                    stop=(t == kmax),
                                )
                                if t == 0 and prev_stop[jj] is not None:
                                    add_dep_helper(
                                        mm.ins,
                                        prev_stop[jj].ins,
                                        sync=False,
                                        reason="serialize PSUM-bank accum groups",
                                    )
                                if t == kmax:
                                    prev_stop[jj] = mm
                        for qt in range(4):
                            jj, m = qt // 2, qt % 2
                            rz = smallp.tile([128, 1], f32, tag="rz")
                            nc.vector.reciprocal(
                                rz[:], o_ps[jj][:, DA * m + D : DA * m + D + 1]
                            )
                            o_sb = outp.tile([128, D], f32, tag="o_sb")
                            nc.vector.tensor_scalar_mul(
                                o_sb[:], o_ps[jj][:, DA * m : DA * m + D], rz[:]
                            )
                            nc.sync.dma_start(
                                out[b, 512 * j + 128 * qt : 512 * j + 128 * (qt + 1), :],
                                o_sb[:],
                            )
    _split_multi_waits(nc)
    return nc


def _get_runner(key, nc):
    """Build (once) a reusable jitted SPMD executor for program `nc`.
    Returns run(in_maps) -> list of per-core output dicts."""
    if key in _RUNNER_CACHE:
        return _RUNNER_CACHE[key]

    import jax
    import concourse.mybir as mybir
    from concourse import bass2jax
    from jax.sharding import Mesh, NamedSharding, PartitionSpec
    from jax.experimental.shard_map import shard_map

    bass2jax.install_neuronx_cc_hook()

    partition_name = (
        nc.partition_id_tensor.name if nc.partition_id_tensor else None
    )
    in_names, out_names, out_avals, zero_outs = [], [], [], []
    for alloc in nc.m.functions[0].allocations:
        if not isinstance(alloc, mybir.MemoryLocationSet):
            continue
        name = alloc.memorylocations[0].name
        if alloc.kind == "ExternalInput":
            if name != partition_name:
                in_names.append(name)
        elif alloc.kind == "ExternalOutput":
            shape = tuple(alloc.tensor_shape)
            dtype = mybir.dt.np(alloc.dtype)
            out_names.append(name)
            out_avals.append(jax.core.ShapedArray(shape, dtype))
            zero_outs.append(np.zeros(shape, dtype))
    n_params = len(in_names)
    n_outs = len(out_avals)
    all_in_names = list(in_names) + list(out_names)
    if partition_name is not None:
        all_in_names.append(partition_name)
    donate = tuple(range(n_params, n_params + n_outs))

    def _body(*args):
        operands = list(args)
        if partition_name is not None:
            operands.append(bass2jax.partition_id_tensor())
        outs = bass2jax._bass_exec_p.bind(
            *operands,
            out_avals=tuple(out_avals),
            in_names=tuple(all_in_names),
            out_names=tuple(out_names),
            lowering_input_output_aliases=(),
            sim_require_finite=True,
            sim_require_nnan=True,
            nc=nc,
        )
        return tuple(outs)

    devices = jax.devices()[:N_CORES]
    mesh = Mesh(np.asarray(devices), ("core",))
    in_specs = (PartitionSpec("core"),) * (n_params + n_outs)
    out_specs = (PartitionSpec("core"),) * n_outs
    sharded = jax.jit(
        shard_map(
            _body, mesh=mesh, in_specs=in_specs, out_specs=out_specs, check_rep=False
        ),
        donate_argnums=donate,
        keep_unused=True,
    )
    sharding = NamedSharding(mesh, PartitionSpec("core"))

    state = {"dev_inputs": None}

    def place_inputs(in_maps):
        import jax as _jax

        concat_in = [
            np.concatenate([np.asarray(m[nm]) for m in in_maps], axis=0)
            for nm in in_names
        ]
        state["dev_inputs"] = [
            _jax.device_put(a, sharding) for a in concat_in
        ]

    def run():
        import jax as _jax

        zeros = [
            _jax.device_put(
                np.zeros((N_CORES * z.shape[0], *z.shape[1:]), z.dtype), sharding
            )
            for z in zero_outs
        ]
        out_arrs = sharded(*state["dev_inputs"], *zeros)
        _jax.block_until_ready(out_arrs)
        return out_arrs

    def collect(out_arrs):
        return [
            {
                nm: np.asarray(out_arrs[i]).reshape(
                    N_CORES, *out_avals[i].shape
                )[c]
                for i, nm in enumerate(out_names)
            }
            for c in range(N_CORES)
        ]

    runner = {"place_inputs": place_inputs, "run": run, "collect": collect}
    _RUNNER_CACHE[key] = runner
    return runner


def _prep_inputs(queries, keys, values, valid_lens, fast=True):
    """Host-side shard + layout prep. Returns per-core in_maps."""
    import ml_dtypes

    queries = np.asarray(queries, dtype=np.float32)
    keys = np.asarray(keys, dtype=np.float32)
    values = np.asarray(values, dtype=np.float32)
    if fast:
        values = values.astype(ml_dtypes.bfloat16)
    valid_lens = np.asarray(valid_lens)

    qT = np.ascontiguousarray(queries.transpose(0, 2, 1))  # [B, D, S]
    kTt = np.ascontiguousarray(keys.transpose(0, 2, 1))  # [B, D, S]
    kpos = np.arange(S)
    pad = (kpos[None, :] < valid_lens[:, None]).astype(np.float32)  # [B, S]
    # pad01[b, p, t] = pad[b, 128*t + p]
    pad01 = np.ascontiguousarray(pad.reshape(B, NT, 128).transpose(0, 2, 1))

    in_maps = []
    for c in range(N_CORES):
        sl = slice(c * B_LOC, (c + 1) * B_LOC)
        in_maps.append(
            {
                "qT": qT[sl],
                "kT": kTt[sl],
                "v": values[sl],
                "pad01": pad01[sl],
            }
        )
    return in_maps


def get_compiled(causal: bool, t_pad_start: int, reps: int = 1, fast: bool = True):
    key = (bool(causal), int(t_pad_start), int(reps), bool(fast))
    if key not in _PROGRAM_CACHE:
        _PROGRAM_CACHE[key] = _build_program(*key)
    return key, _get_runner(key, _PROGRAM_CACHE[key])


def kernel(queries, keys, values, valid_lens, causal, _reps=1, _fast=True):
    causal_b = bool(int(np.asarray(causal)))
    valid_lens = np.asarray(valid_lens)
    t_pad_start = min(int(valid_lens.min()) // 128, NT)

    _, runner = get_compiled(causal_b, t_pad_start, _reps, _fast)
    in_maps = _prep_inputs(queries, keys, values, valid_lens, fast=_fast)
    runner["place_inputs"](in_maps)
    results = runner["collect"](runner["run"]())
    return np.concatenate([r["out"] for r in results], axis=0)


# revision 19
# speedup vs baseline: 268.3481x; 268.3481x over previous
"""Bass/Tile Trainium2 kernel for masked dot-product attention.

Problem: B=32 (batch*heads), S=2048, D=128, fp32.
  out = softmax(mask(Q @ K^T / sqrt(D))) @ V
  mask = key-padding (k >= valid_len[b]) OR causal (k > q).

Sharding: batch dim across 8 cores (4 batches/core), no cross-core comm.

Per-core device algorithm (per batch):
  - Q^T, K^T loaded in [D, S] layout (host pre-transposes during sharding).
  - Scores computed transposed, ST[k, q] = K @ Q^T, in 512-wide q blocks,
    k-tiles chunked 3-at-a-time into one PSUM tile [128, 1536].
  - One ACT exp instruction per chunk (scale=1/sqrt(D) fused), PSUM -> SBUF.
  - Causal mask: multiply diagonal 128x128 sub-tiles by a constant 0/1
    triangle; fully-masked sub-tiles are simply skipped in the PV matmuls.
  - Padding mask: per-partition 0/1 multiplier column (k is the partition
    dim in ST layout), one tensor_scalar mul per affected sub-tile.
  - PV: lhsT = P~ slice [k,128q], rhs = V_aug [k, 129] (V with an appended
    ones column) -> PSUM O[q, 0:128] and Z[q] at column 128 in one pass.
  - Epilogue: rz = 1/Z (DVE reciprocal), out = O * rz (tensor_scalar).
"""

import math

import numpy as np

B, S, D = 32, 2048, 128
N_CORES = 8
B_LOC = B // N_CORES  # 4 batches per core
NT = S // 128  # 16 k-tiles per batch
NJ = S // 512  # 4 q-blocks per batch
CHUNK = 3  # k-tiles per score PSUM tile ([128, 1536] = 3 banks)

_PROGRAM_CACHE = {}
_RUNNER_CACHE = {}


def _apply_tile_drain_patch():
    """walrus on this image only accepts 1 sync-wait per instruction; Tile's
    kernel-tail drain attaches every outstanding sem wait to one drain.
    Spill the excess onto dedicated single-wait NOPs (SP is FIFO, so waiting
    right after the drain and before the barrier is equivalent)."""
    import bass_rust
    import concourse.tile as tile
    from concourse.vector_clock import ScopedClock

    if getattr(tile.TileContext, "_drain_patch_applied", False):
        return

    def _patched(self, tick_clock, wait_clock):
        nc = self.nc
        drain_inst = nc.sync.drain()
        wait_clock.add_sem_waits(
            drain_inst.ins, ScopedClock({None: tick_clock.global_clock})
        )
        si = drain_inst.ins.sync_info
        if si is not None and si.on_wait and len(si.on_wait) > 1:
            waits = list(si.on_wait)
            drain_inst.ins.sync_info = bass_rust.SyncInfo(
                on_wait=[waits[0]], on_update=list(si.on_update or [])
            )
            for w in waits[1:]:
                nop = nc.sync.nop()
                nop.ins.sync_info = bass_rust.SyncInfo(on_wait=[w], on_update=[])
        nc.all_engine_barrier()
        assert self.sems is not None
        popped = nc._tile_sem_poison_stack.pop()
        assert popped is self._sem_poison
        nc.clear_and_free_semaphores(list(self.sems.allocated().values()))
        nc.all_engine_barrier()

    tile.TileContext._drain_and_barrier = _patched
    tile.TileContext._drain_patch_applied = True


def _split_multi_waits(nc):
    """walrus on this image accepts only one sync-wait command per
    instruction; Tile emits several. Move excess waits onto same-engine NOPs
    inserted immediately before the instruction (per-engine streams are
    in-order, so this is equivalent)."""
    import bass_rust
    import concourse.mybir as mybir

    for bb in nc.main_func.blocks:
        insts = bb.instructions
        out = []
        for inst in insts:
            si = inst.sync_info
            if si is not None and si.on_wait and len(si.on_wait) > 1:
                waits = list(si.on_wait)
                for w in waits[:-1]:
                    nop = mybir.InstNoOp(
                        name=f"I-{nc.next_id()}", ins=[], outs=[]
                    )
                    nop.engine = inst.engine
                    nop.sync_info = bass_rust.SyncInfo(on_wait=[w], on_update=[])
                    out.append(nop)
                inst.sync_info = bass_rust.SyncInfo(
                    on_wait=[waits[-1]], on_update=list(si.on_update or [])
                )
            out.append(inst)
        insts[:] = out


def _build_program(
    causal: bool, t_pad_start: int, reps: int = 1, fast: bool = True
):
    """Trace the per-core Bass program. t_pad_start: first k-tile index that
    may contain padding-masked keys (host-computed from min(valid_lens)).
    fast=True: QK matmuls in float32r (full-rate PE) and PV in bf16;
    fast=False: everything plain fp32 (4 cyc/row matmuls, max accuracy)."""
    import concourse.bass as bass
    import concourse.mybir as mybir
    import concourse.tile as tile
    from concourse.tile_rust import add_dep_helper

    _apply_tile_drain_patch()

    f32 = mybir.dt.float32
    f32r = mybir.dt.float32r
    QKDT = f32r if fast else f32  # Q/K dtype (f32r = full-rate PE matmul)
    PDT = mybir.dt.bfloat16 if fast else f32  # probs/V dtype
    DA = D + 1  # V augmented with a ones column

    nc = bass.Bass()
    qT = nc.dram_tensor("qT", [B_LOC, D, S], QKDT, kind="ExternalInput")
    kT = nc.dram_tensor("kT", [B_LOC, D, S], QKDT, kind="ExternalInput")
    v = nc.dram_tensor("v", [B_LOC, S, D], PDT, kind="ExternalInput")
    pad01 = nc.dram_tensor("pad01", [B_LOC, 128, NT], f32, kind="ExternalInput")
    out = nc.dram_tensor("out", [B_LOC, S, D], f32, kind="ExternalOutput")

    with tile.TileContext(nc) as tc:
        with (
            tc.tile_pool(name="const", bufs=1) as constp,
            tc.tile_pool(name="io", bufs=2) as iop,
            tc.tile_pool(name="probs", bufs=8) as probp,
            tc.tile_pool(name="outp", bufs=4) as outp,
            tc.tile_pool(name="small", bufs=4) as smallp,
            tc.tile_pool(name="spsum", bufs=2, space="PSUM") as spsum,
            tc.tile_pool(name="opsum", bufs=2, space="PSUM") as opsum,
        ):
            # 0/1 lower triangle: tri[p, c] = 1.0 iff c >= p (keep k <= q)
            tri = constp.tile([128, 128], PDT)
            nc.gpsimd.memset(tri[:], 1.0)
            nc.gpsimd.affine_select(
                out=tri[:],
                in_=tri[:],
                compare_op=mybir.AluOpType.is_ge,
                fill=0.0,
                base=0,
                pattern=[[1, 128]],
                channel_multiplier=-1,
            )

            for _rep in range(reps):
                for b in range(B_LOC):
                    qt_sb = iop.tile([128, S], QKDT, tag="qT")
                    nc.sync.dma_start(qt_sb[:], qT[b])
                    kt_sb = iop.tile([128, S], QKDT, tag="kT")
                    nc.sync.dma_start(kt_sb[:], kT[b])
                    v_sb = iop.tile([128, NT * DA], PDT, tag="v")
                    v3 = v_sb[:].rearrange("p (t d) -> p t d", d=DA)
                    nc.sync.dma_start(
                        v3[:, :, 0:D], v[b].rearrange("(t p) d -> p t d", p=128)
                    )
                    nc.vector.memset(v3[:, :, D : D + 1], 1.0)
                    pad_sb = iop.tile([128, NT], f32, tag="pad")
                    nc.sync.dma_start(pad_sb[:], pad01[b])

                    for j in range(NJ):
                        T = 4 * j + 4 if causal else NT
                        o_ps = [
                            opsum.tile([128, 2 * DA], f32, tag="o", name=f"o_ps{jj}")
                            for jj in range(2)
                        ]
                        p_tiles = {}  # chunk start -> (p_sb, chunk list)
                        for c0 in range(0, T, CHUNK):
                            ch = list(range(c0, min(c0 + CHUNK, T)))
                            L = len(ch)
                            s_ps = spsum.tile([128, L * 512], f32, tag="s")
                            for i, t in enumerate(ch):
                                nc.tensor.matmul(
                                    s_ps[:, 512 * i : 512 * (i + 1)],
                                    kt_sb[:, 128 * t : 128 * (t + 1)],
                                    qt_sb[:, 512 * j : 512 * (j + 1)],
                                    start=True,
                                    stop=True,
                                )
                            p_sb = probp.tile(
                                [128, L * 512], PDT, tag="p", name=f"p_sb{c0}"
                            )
                            nc.scalar.activation(
                                p_sb[:],
                                s_ps[:],
                                mybir.ActivationFunctionType.Exp,
                                scale=float(1.0 / math.sqrt(D)),
                            )
                            for i, t in enumerate(ch):
                                if causal and t >= 4 * j:
                                    r = t - 4 * j
                                    sl = p_sb[
                                        :, 512 * i + 128 * r : 512 * i + 128 * r + 128
                                    ]
                                    nc.vector.tensor_mul(sl, sl, tri[:])
                                if t >= t_pad_start:
                                    sl = p_sb[:, 512 * i : 512 * (i + 1)]
                                    nc.vector.tensor_scalar_mul(
                                        sl, sl, pad_sb[:, t : t + 1]
                                    )
                            p_tiles[c0] = p_sb
                        # PV: one fully-sequential accumulation group per
                        # (bank, column-half) — interleaving groups on one
                        # PSUM bank corrupts has_written state (start=True
                        # clears the whole bank).
                        prev_stop = [None, None]
                        for qt in range(4):
                            jj, m = qt // 2, qt % 2
                            kmax = (4 * j + qt) if causal else (NT - 1)
                            for t in range(kmax + 1):
                                c0 = (t // CHUNK) * CHUNK
                                i = t - c0
                                p_sb = p_tiles[c0]
                                mm = nc.tensor.matmul(
                                    o_ps[jj][:, DA * m : DA * m + DA],
                                    p_sb[
                                        :, 512 * i + 128 * qt : 512 * i + 128 * qt + 128
                                    ],
                                    v_sb[:, DA * t : DA * (t + 1)],
                                    start=(t == 0),
                                    stop=(t == kmax),
                                )
                                if t == 0 and prev_stop[jj] is not None:
                                    add_dep_helper(
                                        mm.ins,
                                        prev_stop[jj].ins,
                                        sync=False,
                                        reason="serialize PSUM-bank accum groups",
                                    )
                                if t == kmax:
                                    prev_stop[jj] = mm
                        for qt in range(4):
                            jj, m = qt // 2, qt % 2
                            rz = smallp.tile([128, 1], f32, tag="rz")
                            nc.vector.reciprocal(
                                rz[:], o_ps[jj][:, DA * m + D : DA * m + D + 1]
                            )
                            o_sb = outp.tile([128, D], f32, tag="o_sb")
                            nc.vector.tensor_scalar_mul(
                                o_sb[:], o_ps[jj][:, DA * m : DA * m + D], rz[:]
                            )
                            nc.sync.dma_start(
                                out[b, 512 * j + 128 * qt : 512 * j + 128 * (qt + 1), :],
                                o_sb[:],
                            )
    _split_multi_waits(nc)
    return nc


def _get_runner(key, nc):
    """Build (once) a reusable jitted SPMD executor for program `nc`.
    Returns run(in_maps) -> list of per-core output dicts."""
    if key in _RUNNER_CACHE:
        return _RUNNER_CACHE[key]

    import jax
    import concourse.mybir as mybir
    from concourse import bass2jax
    from jax.sharding import Mesh, NamedSharding, PartitionSpec
    from jax.experimental.shard_map import shard_map

    bass2jax.install_neuronx_cc_hook()

    partition_name = (
        nc.partition_id_tensor.name if nc.partition_id_tensor else None
    )
    in_names, out_names, out_avals, zero_outs = [], [], [], []
    for alloc in nc.m.functions[0].allocations:
        if not isinstance(alloc, mybir.MemoryLocationSet):
            continue
        name = alloc.memorylocations[0].name
        if alloc.kind == "ExternalInput":
            if name != partition_name:
                in_names.append(name)
        elif alloc.kind == "ExternalOutput":
            shape = tuple(alloc.tensor_shape)
            dtype = mybir.dt.np(alloc.dtype)
            out_names.append(name)
            out_avals.append(jax.core.ShapedArray(shape, dtype))
            zero_outs.append(np.zeros(shape, dtype))
    n_params = len(in_names)
    n_outs = len(out_avals)
    all_in_names = list(in_names) + list(out_names)
    if partition_name is not None:
        all_in_names.append(partition_name)
    donate = tuple(range(n_params, n_params + n_outs))

    def _body(*args):
        operands = list(args)
        if partition_name is not None:
            operands.append(bass2jax.partition_id_tensor())
        outs = bass2jax._bass_exec_p.bind(
            *operands,
            out_avals=tuple(out_avals),
            in_names=tuple(all_in_names),
            out_names=tuple(out_names),
            lowering_input_output_aliases=(),
            sim_require_finite=True,
            sim_require_nnan=True,
            nc=nc,
        )
        return tuple(outs)

    devices = jax.devices()[:N_CORES]
    mesh = Mesh(np.asarray(devices), ("core",))
    in_specs = (PartitionSpec("core"),) * (n_params + n_outs)
    out_specs = (PartitionSpec("core"),) * n_outs
    # No donation: the kernel writes every output element, so uninitialized
    # custom-call result buffers are fine and the zero "output seed" buffers
    # can stay device-resident and be reused across timed calls.
    sharded = jax.jit(
        shard_map(
            _body, mesh=mesh, in_specs=in_specs, out_specs=out_specs, check_rep=False
        ),
        keep_unused=True,
    )
    sharding = NamedSharding(mesh, PartitionSpec("core"))

    state = {"dev_inputs": None, "dev_zeros": None}

    def place_inputs(in_maps):
        import jax as _jax

        concat_in = [
            np.concatenate([np.asarray(m[nm]) for m in in_maps], axis=0)
            for nm in in_names
        ]
        state["dev_inputs"] = [
            _jax.device_put(a, sharding) for a in concat_in
        ]
        state["dev_zeros"] = [
            _jax.device_put(
                np.zeros((N_CORES * z.shape[0], *z.shape[1:]), z.dtype), sharding
            )
            for z in zero_outs
        ]

    def run():
        import jax as _jax

        out_arrs = sharded(*state["dev_inputs"], *state["dev_zeros"])
        _jax.block_until_ready(out_arrs)
        return out_arrs

    def run_async():
        return sharded(*state["dev_inputs"], *state["dev_zeros"])

    def collect(out_arrs):
        return [
            {
                nm: np.asarray(out_arrs[i]).reshape(
                    N_CORES, *out_avals[i].shape
                )[c]
                for i, nm in enumerate(out_names)
            }
            for c in range(N_CORES)
        ]

    runner = {
        "place_inputs": place_inputs,
        "run": run,
        "run_async": run_async,
        "collect": collect,
    }
    _RUNNER_CACHE[key] = runner
    return runner


def _prep_inputs(queries, keys, values, valid_lens, fast=True):
    """Host-side shard + layout prep. Returns per-core in_maps."""
    import ml_dtypes

    queries = np.asarray(queries, dtype=np.float32)
    keys = np.asarray(keys, dtype=np.float32)
    values = np.asarray(values, dtype=np.float32)
    if fast:
        values = values.astype(ml_dtypes.bfloat16)
    valid_lens = np.asarray(valid_lens)

    qT = np.ascontiguousarray(queries.transpose(0, 2, 1))  # [B, D, S]
    kTt = np.ascontiguousarray(keys.transpose(0, 2, 1))  # [B, D, S]
    kpos = np.arange(S)
    pad = (kpos[None, :] < valid_lens[:, None]).astype(np.float32)  # [B, S]
    # pad01[b, p, t] = pad[b, 128*t + p]
    pad01 = np.ascontiguousarray(pad.reshape(B, NT, 128).transpose(0, 2, 1))

    in_maps = []
    for c in range(N_CORES):
        sl = slice(c * B_LOC, (c + 1) * B_LOC)
        in_maps.append(
            {
                "qT": qT[sl],
                "kT": kTt[sl],
                "v": values[sl],
                "pad01": pad01[sl],
            }
        )
    return in_maps


def get_compiled(causal: bool, t_pad_start: int, reps: int = 1, fast: bool = True):
    key = (bool(causal), int(t_pad_start), int(reps), bool(fast))
    if key not in _PROGRAM_CACHE:
        _PROGRAM_CACHE[key] = _build_program(*key)
    return key, _get_runner(key, _PROGRAM_CACHE[key])


def kernel(queries, keys, values, valid_lens, causal, _reps=1, _fast=True):
    causal_b = bool(int(np.asarray(causal)))
    valid_lens = np.asarray(valid_lens)
    t_pad_start = min(int(valid_lens.min()) // 128, NT)

    _, runner = get_compiled(causal_b, t_pad_start, _reps, _fast)
    in_maps = _prep_inputs(queries, keys, values, valid_lens, fast=_fast)
    runner["place_inputs"](in_maps)
    results = runner["collect"](runner["run"]())
    return np.concatenate([r["out"] for r in results], axis=0)


# revision 24
# speedup vs baseline: 519.3475x; 1.9354x over previous
"""Bass/Tile Trainium2 kernel for masked dot-product attention.

Problem: B=32 (batch*heads), S=2048, D=128, fp32.
  out = softmax(mask(Q @ K^T / sqrt(D))) @ V
  mask = key-padding (k >= valid_len[b]) OR causal (k > q).

Sharding: batch dim across 8 cores (4 batches/core), no cross-core comm.

Per-core device algorithm (per batch):
  - Q^T, K^T loaded in [D, S] layout (host pre-transposes during sharding).
  - Scores computed transposed, ST[k, q] = K @ Q^T, in 512-wide q blocks,
    k-tiles chunked 3-at-a-time into one PSUM tile [128, 1536].
  - One ACT exp instruction per chunk (scale=1/sqrt(D) fused), PSUM -> SBUF.
  - Causal mask: multiply diagonal 128x128 sub-tiles by a constant 0/1
    triangle; fully-masked sub-tiles are simply skipped in the PV matmuls.
  - Padding mask: per-partition 0/1 multiplier column (k is the partition
    dim in ST layout), one tensor_scalar mul per affected sub-tile.
  - PV: lhsT = P~ slice [k,128q], rhs = V_aug [k, 129] (V with an appended
    ones column) -> PSUM O[q, 0:128] and Z[q] at column 128 in one pass.
  - Epilogue: rz = 1/Z (DVE reciprocal), out = O * rz (tensor_scalar).
"""

import math

import numpy as np

B, S, D = 32, 2048, 128
N_CORES = 8
B_LOC = B // N_CORES  # 4 batches per core
NT = S // 128  # 16 k-tiles per batch
NJ = S // 512  # 4 q-blocks per batch
CHUNK = 3  # k-tiles per score PSUM tile ([128, 1536] = 3 banks)

_PROGRAM_CACHE = {}
_RUNNER_CACHE = {}


def _apply_tile_drain_patch():
    """walrus on this image only accepts 1 sync-wait per instruction; Tile's
    kernel-tail drain attaches every outstanding sem wait to one drain.
    Spill the excess onto dedicated single-wait NOPs (SP is FIFO, so waiting
    right after the drain and before the barrier is equivalent)."""
    import bass_rust
    import concourse.tile as tile
    from concourse.vector_clock import ScopedClock

    if getattr(tile.TileContext, "_drain_patch_applied", False):
        return

    def _patched(self, tick_clock, wait_clock):
        nc = self.nc
        drain_inst = nc.sync.drain()
        wait_clock.add_sem_waits(
            drain_inst.ins, ScopedClock({None: tick_clock.global_clock})
        )
        si = drain_inst.ins.sync_info
        if si is not None and si.on_wait and len(si.on_wait) > 1:
            waits = list(si.on_wait)
            drain_inst.ins.sync_info = bass_rust.SyncInfo(
                on_wait=[waits[0]], on_update=list(si.on_update or [])
            )
            for w in waits[1:]:
                nop = nc.sync.nop()
                nop.ins.sync_info = bass_rust.SyncInfo(on_wait=[w], on_update=[])
        nc.all_engine_barrier()
        assert self.sems is not None
        popped = nc._tile_sem_poison_stack.pop()
        assert popped is self._sem_poison
        nc.clear_and_free_semaphores(list(self.sems.allocated().values()))
        nc.all_engine_barrier()

    tile.TileContext._drain_and_barrier = _patched
    tile.TileContext._drain_patch_applied = True


def _split_multi_waits(nc):
    """walrus on this image accepts only one sync-wait command per
    instruction; Tile emits several. Move excess waits onto same-engine NOPs
    inserted immediately before the instruction (per-engine streams are
    in-order, so this is equivalent)."""
    import bass_rust
    import concourse.mybir as mybir

    for bb in nc.main_func.blocks:
        insts = bb.instructions
        out = []
        for inst in insts:
            si = inst.sync_info
            if si is not None and si.on_wait and len(si.on_wait) > 1:
                waits = list(si.on_wait)
                for w in waits[:-1]:
                    nop = mybir.InstNoOp(
                        name=f"I-{nc.next_id()}", ins=[], outs=[]
                    )
                    nop.engine = inst.engine
                    nop.sync_info = bass_rust.SyncInfo(on_wait=[w], on_update=[])
                    out.append(nop)
                inst.sync_info = bass_rust.SyncInfo(
                    on_wait=[waits[-1]], on_update=list(si.on_update or [])
                )
            out.append(inst)
        insts[:] = out


def _build_program(
    causal: bool, t_pad_start: int, reps: int = 1, fast: bool = True
):
    """Trace the per-core Bass program. t_pad_start: first k-tile index that
    may contain padding-masked keys (host-computed from min(valid_lens)).
    fast=True: QK matmuls in float32r (full-rate PE) and PV in bf16;
    fast=False: everything plain fp32 (4 cyc/row matmuls, max accuracy)."""
    import concourse.bass as bass
    import concourse.mybir as mybir
    import concourse.tile as tile
    from concourse.tile_rust import add_dep_helper

    _apply_tile_drain_patch()

    f32 = mybir.dt.float32
    f32r = mybir.dt.float32r
    QKDT = f32r if fast else f32  # Q/K dtype (f32r = full-rate PE matmul)
    PDT = mybir.dt.bfloat16 if fast else f32  # probs/V dtype
    DA = D + 1  # V augmented with a ones column

    nc = bass.Bass()
    qT = nc.dram_tensor("qT", [B_LOC, D, S], QKDT, kind="ExternalInput")
    kT = nc.dram_tensor("kT", [B_LOC, D, S], QKDT, kind="ExternalInput")
    v = nc.dram_tensor("v", [B_LOC, S, D], PDT, kind="ExternalInput")
    pad01 = nc.dram_tensor("pad01", [B_LOC, 128, NT], f32, kind="ExternalInput")
    out = nc.dram_tensor("out", [B_LOC, S, D], f32, kind="ExternalOutput")

    with tile.TileContext(nc) as tc:
        with (
            tc.tile_pool(name="const", bufs=1) as constp,
            tc.tile_pool(name="io", bufs=2) as iop,
            tc.tile_pool(name="probs", bufs=8) as probp,
            tc.tile_pool(name="outp", bufs=4) as outp,
            tc.tile_pool(name="small", bufs=4) as smallp,
            tc.tile_pool(name="spsum", bufs=2, space="PSUM") as spsum,
            tc.tile_pool(name="opsum", bufs=2, space="PSUM") as opsum,
        ):
            # 0/1 lower triangle: tri[p, c] = 1.0 iff c >= p (keep k <= q)
            tri = constp.tile([128, 128], PDT)
            nc.gpsimd.memset(tri[:], 1.0)
            nc.gpsimd.affine_select(
                out=tri[:],
                in_=tri[:],
                compare_op=mybir.AluOpType.is_ge,
                fill=0.0,
                base=0,
                pattern=[[1, 128]],
                channel_multiplier=-1,
            )

            for _rep in range(reps):
                for b in range(B_LOC):
                    # Split the big loads so the first QK chunk can start
                    # after ~0.5 MB instead of the full 3 MB.
                    pad_sb = iop.tile([128, NT], f32, tag="pad")
                    nc.sync.dma_start(pad_sb[:], pad01[b])
                    kt_sb = iop.tile([128, S], QKDT, tag="kT")
                    qt_sb = iop.tile([128, S], QKDT, tag="qT")
                    for h in range(2):
                        sl = slice(1024 * h, 1024 * (h + 1))
                        nc.sync.dma_start(kt_sb[:, sl], kT[b][:, sl])
                        nc.sync.dma_start(qt_sb[:, sl], qT[b][:, sl])
                    v_sb = iop.tile([128, NT * DA], PDT, tag="v")
                    v3 = v_sb[:].rearrange("p (t d) -> p t d", d=DA)
                    nc.sync.dma_start(
                        v3[:, :, 0:D], v[b].rearrange("(t p) d -> p t d", p=128)
                    )
                    nc.vector.memset(v3[:, :, D : D + 1], 1.0)

                    # Last batch runs its q-blocks largest-first so the
                    # kernel tail ends on the smallest PV chain.
                    j_order = (
                        range(NJ - 1, -1, -1) if b == B_LOC - 1 else range(NJ)
                    )
                    for j in j_order:
                        T = 4 * j + 4 if causal else NT
                        o_ps = [
                            opsum.tile([128, 2 * DA], f32, tag="o", name=f"o_ps{jj}")
                            for jj in range(2)
                        ]
                        p_tiles = {}  # chunk start -> (p_sb, chunk list)
                        for c0 in range(0, T, CHUNK):
                            ch = list(range(c0, min(c0 + CHUNK, T)))
                            L = len(ch)
                            s_ps = spsum.tile([128, L * 512], f32, tag="s")
                            for i, t in enumerate(ch):
                                nc.tensor.matmul(
                                    s_ps[:, 512 * i : 512 * (i + 1)],
                                    kt_sb[:, 128 * t : 128 * (t + 1)],
                                    qt_sb[:, 512 * j : 512 * (j + 1)],
                                    start=True,
                                    stop=True,
                                )
                            p_sb = probp.tile(
                                [128, L * 512], PDT, tag="p", name=f"p_sb{c0}"
                            )
                            nc.scalar.activation(
                                p_sb[:],
                                s_ps[:],
                                mybir.ActivationFunctionType.Exp,
                                scale=float(1.0 / math.sqrt(D)),
                            )
                            for i, t in enumerate(ch):
                                if causal and t >= 4 * j:
                                    r = t - 4 * j
                                    sl = p_sb[
                                        :, 512 * i + 128 * r : 512 * i + 128 * r + 128
                                    ]
                                    nc.vector.tensor_mul(sl, sl, tri[:])
                                if t >= t_pad_start:
                                    sl = p_sb[:, 512 * i : 512 * (i + 1)]
                                    nc.vector.tensor_scalar_mul(
                                        sl, sl, pad_sb[:, t : t + 1]
                                    )
                            p_tiles[c0] = p_sb
                        # PV: one fully-sequential accumulation group per
                        # (bank, column-half) — interleaving groups on one
                        # PSUM bank corrupts has_written state (start=True
                        # clears the whole bank).
                        prev_stop = [None, None]
                        for qt in range(4):
                            jj, m = qt // 2, qt % 2
                            kmax = (4 * j + qt) if causal else (NT - 1)
                            for t in range(kmax + 1):
                                c0 = (t // CHUNK) * CHUNK
                                i = t - c0
                                p_sb = p_tiles[c0]
                                mm = nc.tensor.matmul(
                                    o_ps[jj][:, DA * m : DA * m + DA],
                                    p_sb[
                                        :, 512 * i + 128 * qt : 512 * i + 128 * qt + 128
                                    ],
                                    v_sb[:, DA * t : DA * (t + 1)],
                                    start=(t == 0),
                                    stop=(t == kmax),
                                )
                                if t == 0 and prev_stop[jj] is not None:
                                    add_dep_helper(
                                        mm.ins,
                                        prev_stop[jj].ins,
                                        sync=False,
                                        reason="serialize PSUM-bank accum groups",
                                    )
                                if t == kmax:
                                    prev_stop[jj] = mm
                        o_sb = outp.tile([128, 4 * D], f32, tag="o_sb")
                        for qt in range(4):
                            jj, m = qt // 2, qt % 2
                            rz = smallp.tile([128, 1], f32, tag="rz")
                            nc.vector.reciprocal(
                                rz[:], o_ps[jj][:, DA * m + D : DA * m + D + 1]
                            )
                            nc.vector.tensor_scalar_mul(
                                o_sb[:, D * qt : D * (qt + 1)],
                                o_ps[jj][:, DA * m : DA * m + D],
                                rz[:],
                            )
                        # one store per q-block: SBUF [p, (qt, d)] ->
                        # DRAM out[b, 512j + 128qt + p, d]
                        nc.sync.dma_start(
                            out[b, 512 * j : 512 * (j + 1), :].rearrange(
                                "(qt p) d -> p qt d", p=128
                            ),
                            o_sb[:].rearrange("p (qt d) -> p qt d", d=D),
                        )
    _split_multi_waits(nc)
    return nc


def _get_runner(key, nc):
    """Build (once) a reusable jitted SPMD executor for program `nc`.
    Returns run(in_maps) -> list of per-core output dicts."""
    if key in _RUNNER_CACHE:
        return _RUNNER_CACHE[key]

    import jax
    import concourse.mybir as mybir
    from concourse import bass2jax
    from jax.sharding import Mesh, NamedSharding, PartitionSpec
    from jax.experimental.shard_map import shard_map

    bass2jax.install_neuronx_cc_hook()

    partition_name = (
        nc.partition_id_tensor.name if nc.partition_id_tensor else None
    )
    in_names, out_names, out_avals, zero_outs = [], [], [], []
    for alloc in nc.m.functions[0].allocations:
        if not isinstance(alloc, mybir.MemoryLocationSet):
            continue
        name = alloc.memorylocations[0].name
        if alloc.kind == "ExternalInput":
            if name != partition_name:
                in_names.append(name)
        elif alloc.kind == "ExternalOutput":
            shape = tuple(alloc.tensor_shape)
            dtype = mybir.dt.np(alloc.dtype)
            out_names.append(name)
            out_avals.append(jax.core.ShapedArray(shape, dtype))
            zero_outs.append(np.zeros(shape, dtype))
    n_params = len(in_names)
    n_outs = len(out_avals)
    all_in_names = list(in_names) + list(out_names)
    if partition_name is not None:
        all_in_names.append(partition_name)
    donate = tuple(range(n_params, n_params + n_outs))

    def _body(*args):
        operands = list(args)
        if partition_name is not None:
            operands.append(bass2jax.partition_id_tensor())
        outs = bass2jax._bass_exec_p.bind(
            *operands,
            out_avals=tuple(out_avals),
            in_names=tuple(all_in_names),
            out_names=tuple(out_names),
            lowering_input_output_aliases=(),
            sim_require_finite=True,
            sim_require_nnan=True,
            nc=nc,
        )
        return tuple(outs)

    devices = jax.devices()[:N_CORES]
    mesh = Mesh(np.asarray(devices), ("core",))
    in_specs = (PartitionSpec("core"),) * (n_params + n_outs)
    out_specs = (PartitionSpec("core"),) * n_outs
    # No donation: the kernel writes every output element, so uninitialized
    # custom-call result buffers are fine and the zero "output seed" buffers
    # can stay device-resident and be reused across timed calls.
    sharded = jax.jit(
        shard_map(
            _body, mesh=mesh, in_specs=in_specs, out_specs=out_specs, check_rep=False
        ),
        keep_unused=True,
    )
    sharding = NamedSharding(mesh, PartitionSpec("core"))

    state = {"dev_inputs": None, "dev_zeros": None}

    def place_inputs(in_maps):
        import jax as _jax

        concat_in = [
            np.concatenate([np.asarray(m[nm]) for m in in_maps], axis=0)
            for nm in in_names
        ]
        state["dev_inputs"] = [
            _jax.device_put(a, sharding) for a in concat_in
        ]
        state["dev_zeros"] = [
            _jax.device_put(
                np.zeros((N_CORES * z.shape[0], *z.shape[1:]), z.dtype), sharding
            )
            for z in zero_outs
        ]

    def run():
        import jax as _jax

        out_arrs = sharded(*state["dev_inputs"], *state["dev_zeros"])
        _jax.block_until_ready(out_arrs)
        return out_arrs

    def run_async():
        return sharded(*state["dev_inputs"], *state["dev_zeros"])

    def collect(out_arrs):
        return [
            {
                nm: np.asarray(out_arrs[i]).reshape(
                    N_CORES, *out_avals[i].shape
                )[c]
                for i, nm in enumerate(out_names)
            }
            for c in range(N_CORES)
        ]

    runner = {
        "place_inputs": place_inputs,
        "run": run,
        "run_async": run_async,
        "collect": collect,
    }
    _RUNNER_CACHE[key] = runner
    return runner


def _prep_inputs(queries, keys, values, valid_lens, fast=True):
    """Host-side shard + layout prep. Returns per-core in_maps."""
    import ml_dtypes

    queries = np.asarray(queries, dtype=np.float32)
    keys = np.asarray(keys, dtype=np.float32)
    values = np.asarray(values, dtype=np.float32)
    if fast:
        values = values.astype(ml_dtypes.bfloat16)
    valid_lens = np.asarray(valid_lens)

    qT = np.ascontiguousarray(queries.transpose(0, 2, 1))  # [B, D, S]
    kTt = np.ascontiguousarray(keys.transpose(0, 2, 1))  # [B, D, S]
    kpos = np.arange(S)
    pad = (kpos[None, :] < valid_lens[:, None]).astype(np.float32)  # [B, S]
    # pad01[b, p, t] = pad[b, 128*t + p]
    pad01 = np.ascontiguousarray(pad.reshape(B, NT, 128).transpose(0, 2, 1))

    in_maps = []
    for c in range(N_CORES):
        sl = slice(c * B_LOC, (c + 1) * B_LOC)
        in_maps.append(
            {
                "qT": qT[sl],
                "kT": kTt[sl],
                "v": values[sl],
                "pad01": pad01[sl],
            }
        )
    return in_maps


def get_compiled(causal: bool, t_pad_start: int, reps: int = 1, fast: bool = True):
    key = (bool(causal), int(t_pad_start), int(reps), bool(fast))
    if key not in _PROGRAM_CACHE:
        _PROGRAM_CACHE[key] = _build_program(*key)
    return key, _get_runner(key, _PROGRAM_CACHE[key])


def kernel(queries, keys, values, valid_lens, causal, _reps=1, _fast=True):
    causal_b = bool(int(np.asarray(causal)))
    valid_lens = np.asarray(valid_lens)
    t_pad_start = min(int(valid_lens.min()) // 128, NT)

    _, runner = get_compiled(causal_b, t_pad_start, _reps, _fast)
    in_maps = _prep_inputs(queries, keys, values, valid_lens, fast=_fast)
    runner["place_inputs"](in_maps)
    results = runner["collect"](runner["run"]())
    return np.concatenate([r["out"] for r in results], axis=0)
